# revision 22
# baseline (speedup 1.0000x reference)
"""Trainium2 Bass kernel for nn_AngularSymmetry (ANI-style angular symmetry function).

reference math (per molecule b, atoms i,j,k in 0..N-1):
    theta'  = dot(c_i-c_j, c_i-c_k) / (d_ij*d_ik + eps)
    out[b,i] = 2^(1-zeta) * sum_jk (1+cos theta')^zeta
               * exp(-(d_ij^2+d_ik^2+d_jk^2)) * dc_ij*dc_ik*dc_jk

Identities:
    dot(c_i-c_j, c_i-c_k) = S_ii - S_ij - S_ik + S_jk,  S = C C^T (host Gram)
    weight = G_ij*G_ik*G_jk,  G = exp(-d^2)*dc  (host-precomputed NxN)
    (1+cos t)^0.8 = 2^0.8 * exp(1.6*ln|cos(t/2)|)

Sharding: data-parallel over batch, 2 molecules/core, 8 cores. The host does
O(N^2) layout/precompute (Gram matrix, ln G, packed matmul masters); the
device does all O(N^3) work.

Tiles are [i=partition(128), (q,k)=free(512)]; chunk g covers j in
{g, g+32, g+64, g+96}.  Per molecule, three phases (batches ACT table sets):
  ph1 (per chunk): theta = ONE K=8 fp32 matmul (host-packed lhsT/rhs:
         delta-rows give S_ii-S_ij, ones x SflatR gives S_jk,
         (-cT) x cTrep gives -S_ik)  -> PSUM
       denom = 4pi*(d_ij*d_ik+eps)   (GPSIMD tensor_scalar per j)
       recipT = ~1/denom             (DVE reciprocal_approx_fast)
       dlt = |t - round(t)|, t = theta*recipT + 1/4  (custom DVE op) -> fp16
  ph2: c2a = Sin(2pi*dlt) = |cos(theta'/2)|  (trig table, whole molecule)
  ph3 (per chunk): lnw PSUM = bf16 matmuls [ones x lnGflatR + lnG^T x IDrep]
       W = Exp(lnw) = G_ik*G_jk ; l2 = Ln(c2a); p8 = Exp(1.6*l2 + ln2)
       z_j = sum_k p8*W  (tensor_tensor_reduce per j) -> Z[:, j]
  epilogue: out_i = sum_j Z[i,j]*G[i,j]
"""

import numpy as np

# ---- hardcoded problem shape (from spec) ----
B, N = 16, 128
NCORES = 8
MPC = B // NCORES            # molecules per core = 2
EPS = 1e-5
FOURPI = float(4.0 * np.pi)
FOURPI_EPS = float(4.0 * np.pi * EPS)
LN2 = float(np.log(2.0))
MAGIC = 12582912.0           # 1.5*2^23: fp32 round-to-nearest-int magic
TWO_PI_DOWN = float(np.nextafter(np.float32(2.0 * np.pi), np.float32(0.0)))
CHUNK_J = 4                  # j's per PSUM chunk -> mm N = 512
NCHUNK = N // CHUNK_J        # 32
LGCLAMP = -60.0              # clamp for ln(G) (guards dc==0 -> -inf)
KTH = 8                      # K of the fused theta matmul

CFG = {
    "use_custom_round": True,
    "denom_engine": "gpsimd",  # "gpsimd" | "vector"
    "dlt_dtype": "float16",
    "c2a_dtype": "bfloat16",
    "sin_block": 8,            # chunks per Sin instruction
}

_ROUND_OP = None
_GRAPH = None


def _make_round_op():
    """Fused range reduction: out = |t - round(t)|, t = in0*in1 + 0.25.
    Sin(2pi*out) then yields |cos(x)| for x = in0*in1 in turns*2pi."""
    global _ROUND_OP
    if _ROUND_OP is not None:
        return _ROUND_OP
    from concourse import dve_ops
    from concourse.dve_ops import DveOp
    from concourse.dve_spec import C0, C1, Spec, Src0, Src1, Zero, lower, maxx
    from concourse.dve_uop import DveOpSpec

    name = "ANGSYM_RND"
    for op in dve_ops.OPS:
        if op.name == name:
            _ROUND_OP = op
            return op

    tau = Src0 * Src1 + C0
    k = (tau + C1) - C1
    d = tau - k
    body = maxx(d, Zero - d)

    def _ref(in0, in1, s0, s1, imm2):
        f32 = np.float32
        tau = (in0.astype(f32) * in1.astype(f32) + f32(s0)).astype(f32)
        t2 = (tau + f32(s1)).astype(f32)
        kk = (t2 - f32(s1)).astype(f32)
        return np.abs((tau - kk).astype(f32))

    spec = Spec(body=body, reference=_ref)
    opcode = max(dve_ops._SUB_OPCODE_FOR_NAME.values()) + 1
    assert opcode < 0x20
    dve_ops._SUB_OPCODE_FOR_NAME[name] = opcode
    shas = {}
    for ver in ("v3", "v4"):
        try:
            uops = lower(spec, ver=ver)
            shas[ver] = DveOpSpec(
                name=name, opcode=opcode, uops=uops, rd1_en=True
            ).sha(ver)
        except Exception:
            pass
    assert shas, "ANGSYM_RND failed to lower for all DVE versions"
    op = DveOp(name, spec, subdim=False, uops_sha=shas)
    dve_ops.OPS.append(op)
    dve_ops.CUSTOM_DVE_SPECS[name] = spec
    _ROUND_OP = op
    return op


def build_graph(cfg=None):
    """Build the single-core Bass graph (same SPMD graph on all 8 cores)."""
    cfg = dict(CFG, **(cfg or {}))
    from contextlib import ExitStack

    import concourse.bass as bass
    import concourse.tile as tile
    from concourse import bacc, mybir

    f32 = mybir.dt.float32
    bf16 = mybir.dt.bfloat16
    dlt_dt = getattr(mybir.dt, cfg["dlt_dtype"])
    c2a_dt = getattr(mybir.dt, cfg["c2a_dtype"])
    F = mybir.ActivationFunctionType
    ALU = mybir.AluOpType

    round_op = _make_round_op() if cfg["use_custom_round"] else None
    assert round_op is not None, "stock round path removed in v2"

    nc = bacc.Bacc()
    # per-molecule host-precomputed feeds
    d_ext = nc.declare_dram_parameter("d", [MPC, N, N], f32, isOutput=False)
    d4pi_ext = nc.declare_dram_parameter("d4pi", [MPC, N, N], f32, isOutput=False)
    g_ext = nc.declare_dram_parameter("G", [MPC, N, N], f32, isOutput=False)
    lhs_ext = nc.declare_dram_parameter(
        "thlhs", [MPC, KTH, NCHUNK * N], f32, isOutput=False
    )
    rhs_ext = nc.declare_dram_parameter(
        "thrhs", [MPC, KTH, N * N], f32, isOutput=False
    )
    lgt_ext = nc.declare_dram_parameter("lgt", [MPC, N, N], bf16, isOutput=False)
    lgf_ext = nc.declare_dram_parameter("lgf", [MPC, 1, N * N], bf16, isOutput=False)
    idrep_ext = nc.declare_dram_parameter(
        "idrep", [N, CHUNK_J * N], bf16, isOutput=False
    )
    out_ext = nc.declare_dram_parameter("out", [MPC, N], f32, isOutput=True)

    SB = CHUNK_J * N  # 512: chunk width

    with ExitStack() as ctx:
        tc = ctx.enter_context(tile.TileContext(nc))
        consts = ctx.enter_context(tc.tile_pool(name="consts", bufs=1))
        molp = ctx.enter_context(tc.tile_pool(name="mol", bufs=1))
        psum_th = ctx.enter_context(
            tc.tile_pool(name="psum_th", bufs=2, space="PSUM")
        )
        psum_lnw = ctx.enter_context(
            tc.tile_pool(name="psum_lnw", bufs=2, space="PSUM")
        )
        work = ctx.enter_context(tc.tile_pool(name="work", bufs=2))
        scrapp = ctx.enter_context(tc.tile_pool(name="scrap", bufs=4))

        idrep_sb = consts.tile([N, SB], bf16, tag="idrep")
        nc.sync.dma_start(out=idrep_sb[:], in_=idrep_ext[:])
        ones1b = consts.tile([1, N], bf16, tag="ones1b")
        nc.vector.memset(ones1b[:], 1.0)
        ln2c = consts.tile([N, 1], f32, tag="ln2c")
        nc.vector.memset(ln2c[:], LN2)
        tinyc = consts.tile([N, 1], f32, tag="tinyc")
        nc.vector.memset(tinyc[:], 1e-30)

        for m in range(MPC):
            d_sb = molp.tile([N, N], f32, tag="d_sb")
            nc.sync.dma_start(out=d_sb[:], in_=d_ext[m])
            d4pi = molp.tile([N, N], f32, tag="d4pi")
            nc.sync.dma_start(out=d4pi[:], in_=d4pi_ext[m])
            G = molp.tile([N, N], f32, tag="G")
            nc.sync.dma_start(out=G[:], in_=g_ext[m])
            THL = molp.tile([KTH, NCHUNK * N], f32, tag="THL")
            nc.sync.dma_start(out=THL[:], in_=lhs_ext[m])
            THR = molp.tile([KTH, N * N], f32, tag="THR")
            nc.sync.dma_start(out=THR[:], in_=rhs_ext[m])
            LGT = molp.tile([N, N], bf16, tag="LGT")
            nc.sync.dma_start(out=LGT[:], in_=lgt_ext[m])
            LGF = molp.tile([1, N * N], bf16, tag="LGF")
            nc.sync.dma_start(out=LGF[:], in_=lgf_ext[m])

            dlt_all = molp.tile([N, N * N], dlt_dt, tag="dlt_all")
            c2a_all = molp.tile([N, N * N], c2a_dt, tag="c2a_all")
            Z = molp.tile([N, N], f32, tag="Z")

            # ---- phase 1: theta + range-reduce, per chunk ----
            for g in range(NCHUNK):
                js = [g + NCHUNK * q for q in range(CHUNK_J)]
                TH = psum_th.tile([N, SB], f32, tag="TH")
                nc.tensor.matmul(
                    out=TH[:], lhsT=THL[:, g * N:(g + 1) * N],
                    rhs=THR[:, g * SB:(g + 1) * SB], start=True, stop=True,
                )
                denom = work.tile([N, SB], f32, tag="denom")
                dng = nc.gpsimd if cfg["denom_engine"] == "gpsimd" else nc.vector
                for q in range(CHUNK_J):
                    dng.tensor_scalar(
                        out=denom[:, q * N:(q + 1) * N], in0=d_sb[:],
                        scalar1=d4pi[:, js[q]:js[q] + 1],
                        scalar2=FOURPI_EPS, op0=ALU.mult, op1=ALU.add,
                    )
                recipT = work.tile([N, SB], f32, tag="recipT")
                nc.vector.reciprocal_approx_fast(out=recipT[:], in_=denom[:])
                nc.vector._custom_dve(
                    round_op, out=dlt_all[:, g * SB:(g + 1) * SB],
                    in0=TH[:], in1=recipT[:], s0=0.25, s1=MAGIC,
                )

            # ---- phase 2: Sin over the whole molecule (trig table) ----
            sb_blk = cfg["sin_block"] * SB
            for o in range(0, N * N, sb_blk):
                nc.scalar.activation(
                    c2a_all[:, o:o + sb_blk], dlt_all[:, o:o + sb_blk],
                    F.Sin, bias=0.0, scale=TWO_PI_DOWN,
                )

            # ---- phase 3: weights + pow + reduce, per chunk (ln/exp table) --
            for g in range(NCHUNK):
                js = [g + NCHUNK * q for q in range(CHUNK_J)]
                LNW = psum_lnw.tile([N, SB], f32, tag="LNW")
                nc.tensor.matmul(
                    out=LNW[:], lhsT=ones1b[:],
                    rhs=LGF[:, g * SB:(g + 1) * SB], start=True, stop=False,
                )
                nc.tensor.matmul(
                    out=LNW[:], lhsT=LGT[:], rhs=idrep_sb[:],
                    start=False, stop=True,
                )
                W = work.tile([N, SB], f32, tag="W")
                nc.scalar.activation(W[:], LNW[:], F.Exp)
                l2 = work.tile([N, SB], f32, tag="l2")
                nc.scalar.activation(
                    l2[:], c2a_all[:, g * SB:(g + 1) * SB], F.Ln, bias=tinyc[:]
                )
                p8 = work.tile([N, SB], f32, tag="p8")
                nc.scalar.activation(p8[:], l2[:], F.Exp, bias=ln2c[:], scale=1.6)
                for q in range(CHUNK_J):
                    scrap = scrapp.tile([N, N], f32, tag="scrap")
                    nc.vector.affine_mul_reduce(
                        out=scrap[:], accum_out=Z[:, js[q]:js[q] + 1],
                        in0=p8[:, q * N:(q + 1) * N],
                        in1=W[:, q * N:(q + 1) * N],
                        scale=1.0, bias=0.0,
                    )

            # ---- epilogue: out_i = sum_j Z[i,j]*G[i,j] ----
            outc = molp.tile([N, 1], f32, tag="outc")
            escrap = scrapp.tile([N, N], f32, tag="escrap")
            nc.vector.affine_mul_reduce(
                out=escrap[:], accum_out=outc[:], in0=Z[:], in1=G[:],
                scale=1.0, bias=0.0,
            )
            nc.sync.dma_start(out=out_ext[m], in_=outc[:])

    return nc


def _get_graph():
    global _GRAPH
    if _GRAPH is None:
        _GRAPH = build_graph()
        _GRAPH.finalize()
    return _GRAPH


def _host_precompute(d, dc, coords):
    """Per-molecule numpy precompute of the packed device feeds.
    d, dc: [N, N] f32;  coords: [N, 3] f32.  All O(N^2)."""
    import ml_dtypes

    f32 = np.float32
    C = coords.astype(np.float64)
    S = (C @ C.T).astype(f32)                      # Gram
    diag = np.diag(S).copy()
    lnG = np.maximum(
        np.log(dc.astype(np.float64) + 1e-30) - d.astype(np.float64) ** 2,
        LGCLAMP,
    ).astype(f32)
    G = np.exp(lnG).astype(f32)
    cT = coords.T.astype(f32)                      # [3, N]

    # theta matmul masters; chunk g covers js = g + 32*q
    # lhsT slice [KTH, 128] at col g*128; rhs slice [KTH, 512] at col g*512
    THL = np.zeros((KTH, NCHUNK * N), f32)
    THR = np.zeros((KTH, N * N), f32)
    for g in range(NCHUNK):
        li = slice(g * N, (g + 1) * N)
        ri = slice(g * CHUNK_J * N, (g + 1) * CHUNK_J * N)
        lhs = np.zeros((KTH, N), f32)
        rhs = np.zeros((KTH, CHUNK_J * N), f32)
        for q in range(CHUNK_J):
            j = g + NCHUNK * q
            lhs[q, :] = diag - S[:, j]             # S_ii - S_ij  (per i)
            rhs[q, q * N:(q + 1) * N] = 1.0        # delta row
            rhs[4, q * N:(q + 1) * N] = S[j, :]    # S_jk
            rhs[5:8, q * N:(q + 1) * N] = cT       # cTrep
        lhs[4, :] = 1.0                            # ones row (pairs S_jk)
        lhs[5:8, :] = -cT                          # -S_ik
        THL[:, li] = lhs
        THR[:, ri] = rhs

    lgt = np.ascontiguousarray(lnG.T).astype(ml_dtypes.bfloat16)
    # lgf[0, g*512 + q*128 + k] = lnG[g+32q, k]
    lgf = np.zeros((1, N * N), np.float32)
    for q in range(CHUNK_J):
        rows = lnG[NCHUNK * q:NCHUNK * (q + 1), :]          # [32, 128]
        lgf[0].reshape(NCHUNK, CHUNK_J, N)[:, q, :] = rows
    lgf = lgf.astype(ml_dtypes.bfloat16)
    return {
        "d": d.astype(f32),
        "d4pi": (FOURPI * d).astype(f32),
        "G": G,
        "thlhs": THL,
        "thrhs": THR,
        "lgt": lgt,
        "lgf": lgf,
    }


def make_in_maps(d_cutoff, d, atom_coordinates):
    import ml_dtypes

    idrep = np.ascontiguousarray(
        np.tile(np.eye(N, dtype=np.float32), (1, CHUNK_J))
    ).astype(ml_dtypes.bfloat16)
    in_maps = []
    for c in range(NCORES):
        per_mol = [
            _host_precompute(
                np.asarray(d[c * MPC + m], dtype=np.float32),
                np.asarray(d_cutoff[c * MPC + m], dtype=np.float32),
                np.asarray(atom_coordinates[c * MPC + m], dtype=np.float32),
            )
            for m in range(MPC)
        ]
        im = {
            k: np.ascontiguousarray(np.stack([pm[k] for pm in per_mol]))
            for k in per_mol[0]
        }
        im["idrep"] = idrep
        in_maps.append(im)
    return in_maps


def kernel(d_cutoff, d, atom_coordinates):
    from concourse.bass_utils import run_bass_kernel_spmd

    nc = _get_graph()
    in_maps = make_in_maps(d_cutoff, d, atom_coordinates)
    res = run_bass_kernel_spmd(nc, in_maps, list(range(NCORES)))
    out = np.concatenate(
        [res.results[i]["out"] for i in range(NCORES)], axis=0
    ).astype(np.float32)
    return out


# revision 31
# speedup vs baseline: 1.1053x; 1.1053x over previous
"""Trainium2 Bass kernel for nn_AngularSymmetry (ANI-style angular symmetry function).

reference math (per molecule b, atoms i,j,k in 0..N-1):
    theta'  = dot(c_i-c_j, c_i-c_k) / (d_ij*d_ik + eps)
    out[b,i] = 2^(1-zeta) * sum_jk (1+cos theta')^zeta
               * exp(-(d_ij^2+d_ik^2+d_jk^2)) * dc_ij*dc_ik*dc_jk

Identities:
    dot(c_i-c_j, c_i-c_k) = S_ii - S_ij - S_ik + S_jk,  S = C C^T (host Gram)
    weight = G_ij*G_ik*G_jk,  G = exp(-d^2)*dc  (host-precomputed NxN)
    (1+cos t)^0.8 = 2^0.8 * exp(1.6*ln|cos(t/2)|)

Sharding: data-parallel over batch, 2 molecules/core, 8 cores. The host does
O(N^2) layout/precompute (Gram matrix, ln G, packed matmul masters); the
device does all O(N^3) work.

Tiles are [i=partition(128), (q,k)=free(512)]; chunk g covers j in
{g, g+32, g+64, g+96}.  Per molecule, three phases (batches ACT table sets):
  ph1 (per chunk): theta = ONE K=8 fp32 matmul (host-packed lhsT/rhs:
         delta-rows give S_ii-S_ij, ones x SflatR gives S_jk,
         (-cT) x cTrep gives -S_ik)  -> PSUM
       denom = 4pi*(d_ij*d_ik+eps)   (GPSIMD tensor_scalar per j)
       recipT = ~1/denom             (DVE reciprocal_approx_fast)
       dlt = |t - round(t)|, t = theta*recipT + 1/4  (custom DVE op) -> fp16
  ph2: c2a = Sin(2pi*dlt) = |cos(theta'/2)|  (trig table, whole molecule)
  ph3 (per chunk): lnw PSUM = bf16 matmuls [ones x lnGflatR + lnG^T x IDrep]
       W = Exp(lnw) = G_ik*G_jk ; l2 = Ln(c2a); p8 = Exp(1.6*l2 + ln2)
       z_j = sum_k p8*W  (tensor_tensor_reduce per j) -> Z[:, j]
  epilogue: out_i = sum_j Z[i,j]*G[i,j]
"""

import numpy as np

# ---- hardcoded problem shape (from spec) ----
B, N = 16, 128
NCORES = 8
MPC = B // NCORES            # molecules per core = 2
EPS = 1e-5
FOURPI = float(4.0 * np.pi)
FOURPI_EPS = float(4.0 * np.pi * EPS)
LN2 = float(np.log(2.0))
MAGIC = 12582912.0           # 1.5*2^23: fp32 round-to-nearest-int magic
TWO_PI_DOWN = float(np.nextafter(np.float32(2.0 * np.pi), np.float32(0.0)))
CHUNK_J = 4                  # j's per PSUM chunk -> mm N = 512
NCHUNK = N // CHUNK_J        # 32
LGCLAMP = -60.0              # clamp for ln(G) (guards dc==0 -> -inf)
KTH = 8                      # K of the fused theta matmul

CFG = {
    "use_custom_round": True,
    "denom_engine": "gpsimd",  # "gpsimd" | "vector"
    "dlt_dtype": "float16",
    "c2a_dtype": "bfloat16",
    "sin_block": 16,           # chunks per Sin instruction
}

_ROUND_OP = None
_GRAPH = None


def _make_round_op():
    """Fused range reduction: out = |t - round(t)|, t = in0*in1 + 0.25.
    Sin(2pi*out) then yields |cos(x)| for x = in0*in1 in turns*2pi."""
    global _ROUND_OP
    if _ROUND_OP is not None:
        return _ROUND_OP
    from concourse import dve_ops
    from concourse.dve_ops import DveOp
    from concourse.dve_spec import C0, C1, Spec, Src0, Src1, Zero, lower, maxx
    from concourse.dve_uop import DveOpSpec

    name = "ANGSYM_RND"
    for op in dve_ops.OPS:
        if op.name == name:
            _ROUND_OP = op
            return op

    tau = Src0 * Src1 + C0
    k = (tau + C1) - C1
    d = tau - k
    body = maxx(d, Zero - d)

    def _ref(in0, in1, s0, s1, imm2):
        f32 = np.float32
        tau = (in0.astype(f32) * in1.astype(f32) + f32(s0)).astype(f32)
        t2 = (tau + f32(s1)).astype(f32)
        kk = (t2 - f32(s1)).astype(f32)
        return np.abs((tau - kk).astype(f32))

    spec = Spec(body=body, reference=_ref)
    opcode = max(dve_ops._SUB_OPCODE_FOR_NAME.values()) + 1
    assert opcode < 0x20
    dve_ops._SUB_OPCODE_FOR_NAME[name] = opcode
    shas = {}
    for ver in ("v3", "v4"):
        try:
            uops = lower(spec, ver=ver)
            shas[ver] = DveOpSpec(
                name=name, opcode=opcode, uops=uops, rd1_en=True
            ).sha(ver)
        except Exception:
            pass
    assert shas, "ANGSYM_RND failed to lower for all DVE versions"
    op = DveOp(name, spec, subdim=False, uops_sha=shas)
    dve_ops.OPS.append(op)
    dve_ops.CUSTOM_DVE_SPECS[name] = spec
    _ROUND_OP = op
    return op


def build_graph(cfg=None):
    """Build the single-core Bass graph (same SPMD graph on all 8 cores)."""
    cfg = dict(CFG, **(cfg or {}))
    from contextlib import ExitStack

    import concourse.bass as bass
    import concourse.tile as tile
    from concourse import bacc, mybir

    f32 = mybir.dt.float32
    bf16 = mybir.dt.bfloat16
    dlt_dt = getattr(mybir.dt, cfg["dlt_dtype"])
    c2a_dt = getattr(mybir.dt, cfg["c2a_dtype"])
    F = mybir.ActivationFunctionType
    ALU = mybir.AluOpType

    from concourse.tile_rust import add_dep_helper

    round_op = _make_round_op() if cfg["use_custom_round"] else None
    assert round_op is not None, "stock round path removed in v2"

    nc = bacc.Bacc()
    # per-molecule host-precomputed feeds
    d_ext = nc.declare_dram_parameter("d", [MPC, N, N], f32, isOutput=False)
    d4pi_ext = nc.declare_dram_parameter("d4pi", [MPC, N, N], f32, isOutput=False)
    g_ext = nc.declare_dram_parameter("G", [MPC, N, N], f32, isOutput=False)
    lhs_ext = nc.declare_dram_parameter(
        "thl", [MPC, 2 * KTH, NCHUNK * N], bf16, isOutput=False
    )
    rhsa_ext = nc.declare_dram_parameter(
        "thra", [MPC, KTH, N * N], bf16, isOutput=False
    )
    rhsb_ext = nc.declare_dram_parameter(
        "thrb", [MPC, 2 * KTH, N * N], bf16, isOutput=False
    )
    lgt_ext = nc.declare_dram_parameter("lgt", [MPC, N, N], bf16, isOutput=False)
    lgf_ext = nc.declare_dram_parameter("lgf", [MPC, 1, N * N], bf16, isOutput=False)
    idrep_ext = nc.declare_dram_parameter(
        "idrep", [N, CHUNK_J * N], bf16, isOutput=False
    )
    out_ext = nc.declare_dram_parameter("out", [MPC, N], f32, isOutput=True)

    SB = CHUNK_J * N  # 512: chunk width

    with ExitStack() as ctx:
        tc = ctx.enter_context(tile.TileContext(nc))
        consts = ctx.enter_context(tc.tile_pool(name="consts", bufs=1))
        molp = ctx.enter_context(tc.tile_pool(name="mol", bufs=1))
        psum_th = ctx.enter_context(
            tc.tile_pool(name="psum_th", bufs=2, space="PSUM")
        )
        psum_lnw = ctx.enter_context(
            tc.tile_pool(name="psum_lnw", bufs=2, space="PSUM")
        )
        work = ctx.enter_context(tc.tile_pool(name="work", bufs=2))
        scrapp = ctx.enter_context(tc.tile_pool(name="scrap", bufs=4))

        # chain every ACT op in program order so the scheduler cannot
        # interleave trig-table and ln/exp-table phases (each switch costs
        # ~1.5us ACT_TABLE_LOAD; unchained we measured 95 loads = 146us)
        _last_act = [None]

        def act(*a, **kw):
            bi = nc.scalar.activation(*a, **kw)
            if _last_act[0] is not None:
                add_dep_helper(
                    bi.ins, _last_act[0], sync=False, reason="act-table-order"
                )
            _last_act[0] = bi.ins
            return bi

        idrep_sb = consts.tile([N, SB], bf16, tag="idrep")
        nc.sync.dma_start(out=idrep_sb[:], in_=idrep_ext[:])
        ones1b = consts.tile([1, N], bf16, tag="ones1b")
        nc.vector.memset(ones1b[:], 1.0)
        ln2c = consts.tile([N, 1], f32, tag="ln2c")
        nc.vector.memset(ln2c[:], LN2)
        tinyc = consts.tile([N, 1], f32, tag="tinyc")
        nc.vector.memset(tinyc[:], 1e-30)

        for m in range(MPC):
            d_sb = molp.tile([N, N], f32, tag="d_sb")
            nc.sync.dma_start(out=d_sb[:], in_=d_ext[m])
            d4pi = molp.tile([N, N], f32, tag="d4pi")
            nc.sync.dma_start(out=d4pi[:], in_=d4pi_ext[m])
            G = molp.tile([N, N], f32, tag="G")
            nc.sync.dma_start(out=G[:], in_=g_ext[m])
            THL = molp.tile([2 * KTH, NCHUNK * N], bf16, tag="THL")
            nc.sync.dma_start(out=THL[:], in_=lhs_ext[m])
            THRA = molp.tile([KTH, N * N], bf16, tag="THRA")
            nc.sync.dma_start(out=THRA[:], in_=rhsa_ext[m])
            THRB = molp.tile([2 * KTH, N * N], bf16, tag="THRB")
            nc.sync.dma_start(out=THRB[:], in_=rhsb_ext[m])
            LGT = molp.tile([N, N], bf16, tag="LGT")
            nc.sync.dma_start(out=LGT[:], in_=lgt_ext[m])
            LGF = molp.tile([1, N * N], bf16, tag="LGF")
            nc.sync.dma_start(out=LGF[:], in_=lgf_ext[m])

            dlt_all = molp.tile([N, N * N], dlt_dt, tag="dlt_all")
            c2a_all = molp.tile([N, N * N], c2a_dt, tag="c2a_all")
            Z = molp.tile([N, N], f32, tag="Z")

            # ---- phase 1: theta + range-reduce, per chunk ----
            for g in range(NCHUNK):
                js = [g + NCHUNK * q for q in range(CHUNK_J)]
                # theta via split-bf16: hi*hi, then [hi;lo] x [lo;hi]
                TH = psum_th.tile([N, SB], f32, tag="TH")
                nc.tensor.matmul(
                    out=TH[:], lhsT=THL[0:KTH, g * N:(g + 1) * N],
                    rhs=THRA[:, g * SB:(g + 1) * SB], start=True, stop=False,
                )
                nc.tensor.matmul(
                    out=TH[:], lhsT=THL[:, g * N:(g + 1) * N],
                    rhs=THRB[:, g * SB:(g + 1) * SB], start=False, stop=True,
                )
                denom = work.tile([N, SB], f32, tag="denom")
                dng = nc.gpsimd if cfg["denom_engine"] == "gpsimd" else nc.vector
                for q in range(CHUNK_J):
                    dng.tensor_scalar(
                        out=denom[:, q * N:(q + 1) * N], in0=d_sb[:],
                        scalar1=d4pi[:, js[q]:js[q] + 1],
                        scalar2=FOURPI_EPS, op0=ALU.mult, op1=ALU.add,
                    )
                recipT = work.tile([N, SB], f32, tag="recipT")
                nc.vector.reciprocal_approx_fast(out=recipT[:], in_=denom[:])
                nc.vector._custom_dve(
                    round_op, out=dlt_all[:, g * SB:(g + 1) * SB],
                    in0=TH[:], in1=recipT[:], s0=0.25, s1=MAGIC,
                )

            # ---- phase 2: Sin over the whole molecule (trig table) ----
            sb_blk = cfg["sin_block"] * SB
            for o in range(0, N * N, sb_blk):
                act(
                    c2a_all[:, o:o + sb_blk], dlt_all[:, o:o + sb_blk],
                    F.Sin, bias=0.0, scale=TWO_PI_DOWN,
                )

            # ---- phase 3: weights + pow + reduce (ln/exp table) ----
            # Ln/Exp run on 2-chunk blocks; lnw matmuls + ExpW per chunk.
            for gb in range(0, NCHUNK, 2):
                l2 = work.tile([N, 2 * SB], bf16, tag="l2")
                act(
                    l2[:], c2a_all[:, gb * SB:(gb + 2) * SB], F.Ln,
                    bias=tinyc[:],
                )
                p8 = work.tile([N, 2 * SB], f32, tag="p8")
                act(p8[:], l2[:], F.Exp, bias=ln2c[:], scale=1.6)
                for g in (gb, gb + 1):
                    js = [g + NCHUNK * q for q in range(CHUNK_J)]
                    LNW = psum_lnw.tile([N, SB], f32, tag="LNW")
                    nc.tensor.matmul(
                        out=LNW[:], lhsT=ones1b[:],
                        rhs=LGF[:, g * SB:(g + 1) * SB], start=True, stop=False,
                    )
                    nc.tensor.matmul(
                        out=LNW[:], lhsT=LGT[:], rhs=idrep_sb[:],
                        start=False, stop=True,
                    )
                    W = work.tile([N, SB], f32, tag="W")
                    act(W[:], LNW[:], F.Exp)
                    po = (g - gb) * SB
                    for q in range(CHUNK_J):
                        scrap = scrapp.tile([N, N], f32, tag="scrap")
                        nc.vector.affine_mul_reduce(
                            out=scrap[:], accum_out=Z[:, js[q]:js[q] + 1],
                            in0=p8[:, po + q * N:po + (q + 1) * N],
                            in1=W[:, q * N:(q + 1) * N],
                            scale=1.0, bias=0.0,
                        )

            # ---- epilogue: out_i = sum_j Z[i,j]*G[i,j] ----
            outc = molp.tile([N, 1], f32, tag="outc")
            escrap = scrapp.tile([N, N], f32, tag="escrap")
            nc.vector.affine_mul_reduce(
                out=escrap[:], accum_out=outc[:], in0=Z[:], in1=G[:],
                scale=1.0, bias=0.0,
            )
            nc.sync.dma_start(out=out_ext[m], in_=outc[:])

    return nc


def _get_graph():
    global _GRAPH
    if _GRAPH is None:
        _GRAPH = build_graph()
        _GRAPH.finalize()
    return _GRAPH


def _host_precompute(d, dc, coords):
    """Per-molecule numpy precompute of the packed device feeds.
    d, dc: [N, N] f32;  coords: [N, 3] f32.  All O(N^2)."""
    import ml_dtypes

    f32 = np.float32
    C = coords.astype(np.float64)
    S = (C @ C.T).astype(f32)                      # Gram
    diag = np.diag(S).copy()
    lnG = np.maximum(
        np.log(dc.astype(np.float64) + 1e-30) - d.astype(np.float64) ** 2,
        LGCLAMP,
    ).astype(f32)
    G = np.exp(lnG).astype(f32)
    cT = coords.T.astype(f32)                      # [3, N]

    # theta matmul masters; chunk g covers js = g + 32*q
    # lhsT slice at col g*128; rhs slice at col g*512
    THL = np.zeros((KTH, NCHUNK * N), f32)
    THR = np.zeros((KTH, N * N), f32)
    for g in range(NCHUNK):
        li = slice(g * N, (g + 1) * N)
        ri = slice(g * CHUNK_J * N, (g + 1) * CHUNK_J * N)
        lhs = np.zeros((KTH, N), f32)
        rhs = np.zeros((KTH, CHUNK_J * N), f32)
        for q in range(CHUNK_J):
            j = g + NCHUNK * q
            lhs[q, :] = diag - S[:, j]             # S_ii - S_ij  (per i)
            rhs[q, q * N:(q + 1) * N] = 1.0        # delta row
            rhs[4, q * N:(q + 1) * N] = S[j, :]    # S_jk
            rhs[5:8, q * N:(q + 1) * N] = cT       # cTrep
        lhs[4, :] = 1.0                            # ones row (pairs S_jk)
        lhs[5:8, :] = -cT                          # -S_ik
        THL[:, li] = lhs
        THR[:, ri] = rhs

    # split-bf16: a = hi + lo; theta = hi.hi + (hi.lo + lo.hi); the lo.lo
    # term (~2^-16 relative) is dropped.
    def split_bf16(a):
        hi = a.astype(ml_dtypes.bfloat16)
        lo = (a - hi.astype(f32)).astype(ml_dtypes.bfloat16)
        return hi, lo

    THL_hi, THL_lo = split_bf16(THL)
    THR_hi, THR_lo = split_bf16(THR)
    thl = np.concatenate([THL_hi, THL_lo], axis=0)     # [16, 4096] (hi; lo)
    thra = THR_hi                                      # [8, 16384]
    thrb = np.concatenate([THR_lo, THR_hi], axis=0)    # [16, 16384] (lo; hi)

    lgt = np.ascontiguousarray(lnG.T).astype(ml_dtypes.bfloat16)
    # lgf[0, g*512 + q*128 + k] = lnG[g+32q, k]
    lgf = np.zeros((1, N * N), np.float32)
    for q in range(CHUNK_J):
        rows = lnG[NCHUNK * q:NCHUNK * (q + 1), :]          # [32, 128]
        lgf[0].reshape(NCHUNK, CHUNK_J, N)[:, q, :] = rows
    lgf = lgf.astype(ml_dtypes.bfloat16)
    return {
        "d": d.astype(f32),
        "d4pi": (FOURPI * d).astype(f32),
        "G": G,
        "thl": thl,
        "thra": thra,
        "thrb": thrb,
        "lgt": lgt,
        "lgf": lgf,
    }


def make_in_maps(d_cutoff, d, atom_coordinates):
    import ml_dtypes

    idrep = np.ascontiguousarray(
        np.tile(np.eye(N, dtype=np.float32), (1, CHUNK_J))
    ).astype(ml_dtypes.bfloat16)
    in_maps = []
    for c in range(NCORES):
        per_mol = [
            _host_precompute(
                np.asarray(d[c * MPC + m], dtype=np.float32),
                np.asarray(d_cutoff[c * MPC + m], dtype=np.float32),
                np.asarray(atom_coordinates[c * MPC + m], dtype=np.float32),
            )
            for m in range(MPC)
        ]
        im = {
            k: np.ascontiguousarray(np.stack([pm[k] for pm in per_mol]))
            for k in per_mol[0]
        }
        im["idrep"] = idrep
        in_maps.append(im)
    return in_maps


def kernel(d_cutoff, d, atom_coordinates):
    from concourse.bass_utils import run_bass_kernel_spmd

    nc = _get_graph()
    in_maps = make_in_maps(d_cutoff, d, atom_coordinates)
    res = run_bass_kernel_spmd(nc, in_maps, list(range(NCORES)))
    out = np.concatenate(
        [res.results[i]["out"] for i in range(NCORES)], axis=0
    ).astype(np.float32)
    return out


# revision 33
# speedup vs baseline: 1.4685x; 1.3286x over previous
"""Trainium2 Bass kernel for nn_AngularSymmetry (ANI-style angular symmetry function).

reference math (per molecule b, atoms i,j,k in 0..N-1):
    theta'  = dot(c_i-c_j, c_i-c_k) / (d_ij*d_ik + eps)
    out[b,i] = 2^(1-zeta) * sum_jk (1+cos theta')^zeta
               * exp(-(d_ij^2+d_ik^2+d_jk^2)) * dc_ij*dc_ik*dc_jk

Identities:
    dot(c_i-c_j, c_i-c_k) = S_ii - S_ij - S_ik + S_jk,  S = C C^T (host Gram)
    weight = G_ij*G_ik*G_jk,  G = exp(-d^2)*dc  (host-precomputed NxN)
    (1+cos t)^0.8 = 2^0.8 * exp(1.6*ln|cos(t/2)|)

Sharding: data-parallel over batch, 2 molecules/core, 8 cores. The host does
O(N^2) layout/precompute (Gram matrix, ln G, packed matmul masters); the
device does all O(N^3) work.

Tiles are [i=partition(128), (q,k)=free(512)]; chunk g covers j in
{g, g+32, g+64, g+96}.  Per molecule, three phases (batches ACT table sets):
  ph1 (per chunk): theta = ONE K=8 fp32 matmul (host-packed lhsT/rhs:
         delta-rows give S_ii-S_ij, ones x SflatR gives S_jk,
         (-cT) x cTrep gives -S_ik)  -> PSUM
       denom = 4pi*(d_ij*d_ik+eps)   (GPSIMD tensor_scalar per j)
       recipT = ~1/denom             (DVE reciprocal_approx_fast)
       dlt = |t - round(t)|, t = theta*recipT + 1/4  (custom DVE op) -> fp16
  ph2: c2a = Sin(2pi*dlt) = |cos(theta'/2)|  (trig table, whole molecule)
  ph3 (per chunk): lnw PSUM = bf16 matmuls [ones x lnGflatR + lnG^T x IDrep]
       W = Exp(lnw) = G_ik*G_jk ; l2 = Ln(c2a); p8 = Exp(1.6*l2 + ln2)
       z_j = sum_k p8*W  (tensor_tensor_reduce per j) -> Z[:, j]
  epilogue: out_i = sum_j Z[i,j]*G[i,j]
"""

import numpy as np

# ---- hardcoded problem shape (from spec) ----
B, N = 16, 128
NCORES = 8
MPC = B // NCORES            # molecules per core = 2
EPS = 1e-5
FOURPI = float(4.0 * np.pi)
FOURPI_EPS = float(4.0 * np.pi * EPS)
LN2 = float(np.log(2.0))
MAGIC = 12582912.0           # 1.5*2^23: fp32 round-to-nearest-int magic
TWO_PI_DOWN = float(np.nextafter(np.float32(2.0 * np.pi), np.float32(0.0)))
CHUNK_J = 4                  # j's per PSUM chunk -> mm N = 512
NCHUNK = N // CHUNK_J        # 32
LGCLAMP = -60.0              # clamp for ln(G) (guards dc==0 -> -inf)
KTH = 8                      # K of the fused theta matmul

CFG = {
    "use_custom_round": True,
    "denom_engine": "gpsimd",  # "gpsimd" | "vector"
    "dlt_dtype": "float16",
    "c2a_dtype": "bfloat16",
    "sin_block": 16,           # chunks per Sin instruction
}

_ROUND_OP = None
_GRAPH = None


def _make_round_op():
    """Fused range reduction: out = |t - round(t)|, t = in0*in1 + 0.25.
    Sin(2pi*out) then yields |cos(x)| for x = in0*in1 in turns*2pi."""
    global _ROUND_OP
    if _ROUND_OP is not None:
        return _ROUND_OP
    from concourse import dve_ops
    from concourse.dve_ops import DveOp
    from concourse.dve_spec import C0, C1, Spec, Src0, Src1, Zero, lower, maxx
    from concourse.dve_uop import DveOpSpec

    name = "ANGSYM_RND"
    for op in dve_ops.OPS:
        if op.name == name:
            _ROUND_OP = op
            return op

    tau = Src0 * Src1 + C0
    k = (tau + C1) - C1
    d = tau - k
    body = maxx(d, Zero - d)

    def _ref(in0, in1, s0, s1, imm2):
        f32 = np.float32
        tau = (in0.astype(f32) * in1.astype(f32) + f32(s0)).astype(f32)
        t2 = (tau + f32(s1)).astype(f32)
        kk = (t2 - f32(s1)).astype(f32)
        return np.abs((tau - kk).astype(f32))

    spec = Spec(body=body, reference=_ref)
    opcode = max(dve_ops._SUB_OPCODE_FOR_NAME.values()) + 1
    assert opcode < 0x20
    dve_ops._SUB_OPCODE_FOR_NAME[name] = opcode
    shas = {}
    for ver in ("v3", "v4"):
        try:
            uops = lower(spec, ver=ver)
            shas[ver] = DveOpSpec(
                name=name, opcode=opcode, uops=uops, rd1_en=True
            ).sha(ver)
        except Exception:
            pass
    assert shas, "ANGSYM_RND failed to lower for all DVE versions"
    op = DveOp(name, spec, subdim=False, uops_sha=shas)
    dve_ops.OPS.append(op)
    dve_ops.CUSTOM_DVE_SPECS[name] = spec
    _ROUND_OP = op
    return op


def build_graph(cfg=None):
    """Build the single-core Bass graph (same SPMD graph on all 8 cores)."""
    cfg = dict(CFG, **(cfg or {}))
    from contextlib import ExitStack

    import concourse.bass as bass
    import concourse.tile as tile
    from concourse import bacc, mybir

    f32 = mybir.dt.float32
    bf16 = mybir.dt.bfloat16
    dlt_dt = getattr(mybir.dt, cfg["dlt_dtype"])
    c2a_dt = getattr(mybir.dt, cfg["c2a_dtype"])
    F = mybir.ActivationFunctionType
    ALU = mybir.AluOpType

    from concourse.tile_rust import add_dep_helper

    round_op = _make_round_op() if cfg["use_custom_round"] else None
    assert round_op is not None, "stock round path removed in v2"

    nc = bacc.Bacc()
    # per-molecule host-precomputed feeds
    d_ext = nc.declare_dram_parameter("d", [MPC, N, N], f32, isOutput=False)
    d4pi_ext = nc.declare_dram_parameter("d4pi", [MPC, N, N], f32, isOutput=False)
    g_ext = nc.declare_dram_parameter("G", [MPC, N, N], f32, isOutput=False)
    lhs_ext = nc.declare_dram_parameter(
        "thl", [MPC, 2 * KTH, NCHUNK * N], bf16, isOutput=False
    )
    rhsa_ext = nc.declare_dram_parameter(
        "thra", [MPC, KTH, N * N], bf16, isOutput=False
    )
    rhsb_ext = nc.declare_dram_parameter(
        "thrb", [MPC, 2 * KTH, N * N], bf16, isOutput=False
    )
    lgt_ext = nc.declare_dram_parameter("lgt", [MPC, N, N], bf16, isOutput=False)
    lgf_ext = nc.declare_dram_parameter("lgf", [MPC, 1, N * N], bf16, isOutput=False)
    idrep_ext = nc.declare_dram_parameter(
        "idrep", [N, CHUNK_J * N], bf16, isOutput=False
    )
    out_ext = nc.declare_dram_parameter("out", [MPC, N], f32, isOutput=True)

    SB = CHUNK_J * N  # 512: chunk width

    with ExitStack() as ctx:
        tc = ctx.enter_context(tile.TileContext(nc))
        consts = ctx.enter_context(tc.tile_pool(name="consts", bufs=1))
        molp = ctx.enter_context(tc.tile_pool(name="mol", bufs=1))
        psum_th = ctx.enter_context(
            tc.tile_pool(name="psum_th", bufs=2, space="PSUM")
        )
        psum_lnw = ctx.enter_context(
            tc.tile_pool(name="psum_lnw", bufs=2, space="PSUM")
        )
        work = ctx.enter_context(tc.tile_pool(name="work", bufs=2))
        scrapp = ctx.enter_context(tc.tile_pool(name="scrap", bufs=4))

        # chain every ACT op in program order so the scheduler cannot
        # interleave trig-table and ln/exp-table phases (each switch costs
        # ~1.5us ACT_TABLE_LOAD; unchained we measured 95 loads = 146us)
        _last_act = [None]

        def _chain(ins):
            if _last_act[0] is not None:
                add_dep_helper(
                    ins, _last_act[0], sync=False, reason="act-table-order"
                )
            _last_act[0] = ins

        def act(*a, **kw):
            bi = nc.scalar.activation(*a, **kw)
            _chain(bi.ins)
            return bi

        # the auto table-load pass greedily picks per-function sets
        # (natural_log for Ln, exp_and_others for Exp -> one 1.5us load per
        # transition); pre-load the shared ln+exp set explicitly instead.
        from concourse.hw_specs import get_activation_tables

        _tables = get_activation_tables(nc.m.arch)
        _lnexp_id = next(
            i for i, (nm, fs) in enumerate(_tables.items())
            if F.Ln in fs and F.Exp in fs
        )

        def load_lnexp_table():
            inst = mybir.InstLoadActFuncSet(
                name=nc.get_next_instruction_name(), ins=[], outs=[],
                act_func_set_id=_lnexp_id,
            )
            bi = nc.scalar.add_instruction(inst)
            _chain(bi.ins)

        idrep_sb = consts.tile([N, SB], bf16, tag="idrep")
        nc.sync.dma_start(out=idrep_sb[:], in_=idrep_ext[:])
        ones1b = consts.tile([1, N], bf16, tag="ones1b")
        nc.vector.memset(ones1b[:], 1.0)
        ln2c = consts.tile([N, 1], f32, tag="ln2c")
        nc.vector.memset(ln2c[:], LN2)
        tinyc = consts.tile([N, 1], f32, tag="tinyc")
        nc.vector.memset(tinyc[:], 1e-30)

        for m in range(MPC):
            d_sb = molp.tile([N, N], f32, tag="d_sb")
            nc.sync.dma_start(out=d_sb[:], in_=d_ext[m])
            d4pi = molp.tile([N, N], f32, tag="d4pi")
            nc.sync.dma_start(out=d4pi[:], in_=d4pi_ext[m])
            G = molp.tile([N, N], f32, tag="G")
            nc.sync.dma_start(out=G[:], in_=g_ext[m])
            THL = molp.tile([2 * KTH, NCHUNK * N], bf16, tag="THL")
            nc.sync.dma_start(out=THL[:], in_=lhs_ext[m])
            THRA = molp.tile([KTH, N * N], bf16, tag="THRA")
            nc.sync.dma_start(out=THRA[:], in_=rhsa_ext[m])
            THRB = molp.tile([2 * KTH, N * N], bf16, tag="THRB")
            nc.sync.dma_start(out=THRB[:], in_=rhsb_ext[m])
            LGT = molp.tile([N, N], bf16, tag="LGT")
            nc.sync.dma_start(out=LGT[:], in_=lgt_ext[m])
            LGF = molp.tile([1, N * N], bf16, tag="LGF")
            nc.sync.dma_start(out=LGF[:], in_=lgf_ext[m])

            dlt_all = molp.tile([N, N * N], dlt_dt, tag="dlt_all")
            c2a_all = molp.tile([N, N * N], c2a_dt, tag="c2a_all")
            Z = molp.tile([N, N], f32, tag="Z")

            # ---- phase 1: theta + range-reduce, per chunk ----
            for g in range(NCHUNK):
                js = [g + NCHUNK * q for q in range(CHUNK_J)]
                # theta via split-bf16: hi*hi, then [hi;lo] x [lo;hi]
                TH = psum_th.tile([N, SB], f32, tag="TH")
                nc.tensor.matmul(
                    out=TH[:], lhsT=THL[0:KTH, g * N:(g + 1) * N],
                    rhs=THRA[:, g * SB:(g + 1) * SB], start=True, stop=False,
                )
                nc.tensor.matmul(
                    out=TH[:], lhsT=THL[:, g * N:(g + 1) * N],
                    rhs=THRB[:, g * SB:(g + 1) * SB], start=False, stop=True,
                )
                denom = work.tile([N, SB], f32, tag="denom")
                dng = nc.gpsimd if cfg["denom_engine"] == "gpsimd" else nc.vector
                for q in range(CHUNK_J):
                    dng.tensor_scalar(
                        out=denom[:, q * N:(q + 1) * N], in0=d_sb[:],
                        scalar1=d4pi[:, js[q]:js[q] + 1],
                        scalar2=FOURPI_EPS, op0=ALU.mult, op1=ALU.add,
                    )
                recipT = work.tile([N, SB], f32, tag="recipT")
                nc.vector.reciprocal_approx_fast(out=recipT[:], in_=denom[:])
                nc.vector._custom_dve(
                    round_op, out=dlt_all[:, g * SB:(g + 1) * SB],
                    in0=TH[:], in1=recipT[:], s0=0.25, s1=MAGIC,
                )

            # ---- phase 2: Sin over the whole molecule (trig table) ----
            sb_blk = cfg["sin_block"] * SB
            for o in range(0, N * N, sb_blk):
                act(
                    c2a_all[:, o:o + sb_blk], dlt_all[:, o:o + sb_blk],
                    F.Sin, bias=0.0, scale=TWO_PI_DOWN,
                )

            # ---- phase 3: weights + pow + reduce (ln/exp table) ----
            # Ln/Exp run on 2-chunk blocks; lnw matmuls + ExpW per chunk.
            load_lnexp_table()
            for gb in range(0, NCHUNK, 2):
                l2 = work.tile([N, 2 * SB], bf16, tag="l2")
                act(
                    l2[:], c2a_all[:, gb * SB:(gb + 2) * SB], F.Ln,
                    bias=tinyc[:],
                )
                p8 = work.tile([N, 2 * SB], f32, tag="p8")
                act(p8[:], l2[:], F.Exp, bias=ln2c[:], scale=1.6)
                for g in (gb, gb + 1):
                    js = [g + NCHUNK * q for q in range(CHUNK_J)]
                    LNW = psum_lnw.tile([N, SB], f32, tag="LNW")
                    nc.tensor.matmul(
                        out=LNW[:], lhsT=ones1b[:],
                        rhs=LGF[:, g * SB:(g + 1) * SB], start=True, stop=False,
                    )
                    nc.tensor.matmul(
                        out=LNW[:], lhsT=LGT[:], rhs=idrep_sb[:],
                        start=False, stop=True,
                    )
                    W = work.tile([N, SB], f32, tag="W")
                    act(W[:], LNW[:], F.Exp)
                    po = (g - gb) * SB
                    for q in range(CHUNK_J):
                        scrap = scrapp.tile([N, N], f32, tag="scrap")
                        nc.vector.affine_mul_reduce(
                            out=scrap[:], accum_out=Z[:, js[q]:js[q] + 1],
                            in0=p8[:, po + q * N:po + (q + 1) * N],
                            in1=W[:, q * N:(q + 1) * N],
                            scale=1.0, bias=0.0,
                        )

            # ---- epilogue: out_i = sum_j Z[i,j]*G[i,j] ----
            outc = molp.tile([N, 1], f32, tag="outc")
            escrap = scrapp.tile([N, N], f32, tag="escrap")
            nc.vector.affine_mul_reduce(
                out=escrap[:], accum_out=outc[:], in0=Z[:], in1=G[:],
                scale=1.0, bias=0.0,
            )
            nc.sync.dma_start(out=out_ext[m], in_=outc[:])

    return nc


def _get_graph():
    global _GRAPH
    if _GRAPH is None:
        _GRAPH = build_graph()
        _GRAPH.finalize()
    return _GRAPH


def _host_precompute(d, dc, coords):
    """Per-molecule numpy precompute of the packed device feeds.
    d, dc: [N, N] f32;  coords: [N, 3] f32.  All O(N^2)."""
    import ml_dtypes

    f32 = np.float32
    C = coords.astype(np.float64)
    S = (C @ C.T).astype(f32)                      # Gram
    diag = np.diag(S).copy()
    lnG = np.maximum(
        np.log(dc.astype(np.float64) + 1e-30) - d.astype(np.float64) ** 2,
        LGCLAMP,
    ).astype(f32)
    G = np.exp(lnG).astype(f32)
    cT = coords.T.astype(f32)                      # [3, N]

    # theta matmul masters; chunk g covers js = g + 32*q
    # lhsT slice at col g*128; rhs slice at col g*512
    THL = np.zeros((KTH, NCHUNK * N), f32)
    THR = np.zeros((KTH, N * N), f32)
    for g in range(NCHUNK):
        li = slice(g * N, (g + 1) * N)
        ri = slice(g * CHUNK_J * N, (g + 1) * CHUNK_J * N)
        lhs = np.zeros((KTH, N), f32)
        rhs = np.zeros((KTH, CHUNK_J * N), f32)
        for q in range(CHUNK_J):
            j = g + NCHUNK * q
            lhs[q, :] = diag - S[:, j]             # S_ii - S_ij  (per i)
            rhs[q, q * N:(q + 1) * N] = 1.0        # delta row
            rhs[4, q * N:(q + 1) * N] = S[j, :]    # S_jk
            rhs[5:8, q * N:(q + 1) * N] = cT       # cTrep
        lhs[4, :] = 1.0                            # ones row (pairs S_jk)
        lhs[5:8, :] = -cT                          # -S_ik
        THL[:, li] = lhs
        THR[:, ri] = rhs

    # split-bf16: a = hi + lo; theta = hi.hi + (hi.lo + lo.hi); the lo.lo
    # term (~2^-16 relative) is dropped.
    def split_bf16(a):
        hi = a.astype(ml_dtypes.bfloat16)
        lo = (a - hi.astype(f32)).astype(ml_dtypes.bfloat16)
        return hi, lo

    THL_hi, THL_lo = split_bf16(THL)
    THR_hi, THR_lo = split_bf16(THR)
    thl = np.concatenate([THL_hi, THL_lo], axis=0)     # [16, 4096] (hi; lo)
    thra = THR_hi                                      # [8, 16384]
    thrb = np.concatenate([THR_lo, THR_hi], axis=0)    # [16, 16384] (lo; hi)

    lgt = np.ascontiguousarray(lnG.T).astype(ml_dtypes.bfloat16)
    # lgf[0, g*512 + q*128 + k] = lnG[g+32q, k]
    lgf = np.zeros((1, N * N), np.float32)
    for q in range(CHUNK_J):
        rows = lnG[NCHUNK * q:NCHUNK * (q + 1), :]          # [32, 128]
        lgf[0].reshape(NCHUNK, CHUNK_J, N)[:, q, :] = rows
    lgf = lgf.astype(ml_dtypes.bfloat16)
    return {
        "d": d.astype(f32),
        "d4pi": (FOURPI * d).astype(f32),
        "G": G,
        "thl": thl,
        "thra": thra,
        "thrb": thrb,
        "lgt": lgt,
        "lgf": lgf,
    }


def make_in_maps(d_cutoff, d, atom_coordinates):
    import ml_dtypes

    idrep = np.ascontiguousarray(
        np.tile(np.eye(N, dtype=np.float32), (1, CHUNK_J))
    ).astype(ml_dtypes.bfloat16)
    in_maps = []
    for c in range(NCORES):
        per_mol = [
            _host_precompute(
                np.asarray(d[c * MPC + m], dtype=np.float32),
                np.asarray(d_cutoff[c * MPC + m], dtype=np.float32),
                np.asarray(atom_coordinates[c * MPC + m], dtype=np.float32),
            )
            for m in range(MPC)
        ]
        im = {
            k: np.ascontiguousarray(np.stack([pm[k] for pm in per_mol]))
            for k in per_mol[0]
        }
        im["idrep"] = idrep
        in_maps.append(im)
    return in_maps


def kernel(d_cutoff, d, atom_coordinates):
    from concourse.bass_utils import run_bass_kernel_spmd

    nc = _get_graph()
    in_maps = make_in_maps(d_cutoff, d, atom_coordinates)
    res = run_bass_kernel_spmd(nc, in_maps, list(range(NCORES)))
    out = np.concatenate(
        [res.results[i]["out"] for i in range(NCORES)], axis=0
    ).astype(np.float32)
    return out


# revision 35
# speedup vs baseline: 1.6689x; 1.1365x over previous
"""Trainium2 Bass kernel for nn_AngularSymmetry (ANI-style angular symmetry function).

reference math (per molecule b, atoms i,j,k in 0..N-1):
    theta'  = dot(c_i-c_j, c_i-c_k) / (d_ij*d_ik + eps)
    out[b,i] = 2^(1-zeta) * sum_jk (1+cos theta')^zeta
               * exp(-(d_ij^2+d_ik^2+d_jk^2)) * dc_ij*dc_ik*dc_jk

Identities:
    dot(c_i-c_j, c_i-c_k) = S_ii - S_ij - S_ik + S_jk,  S = C C^T (host Gram)
    weight = G_ij*G_ik*G_jk,  G = exp(-d^2)*dc  (host-precomputed NxN)
    (1+cos t)^0.8 = 2^0.8 * exp(1.6*ln|cos(t/2)|)

Sharding: data-parallel over batch, 2 molecules/core, 8 cores. The host does
O(N^2) layout/precompute (Gram matrix, ln G, packed matmul masters); the
device does all O(N^3) work.

Tiles are [i=partition(128), (q,k)=free(512)]; chunk g covers j in
{g, g+32, g+64, g+96}.  Per molecule, three phases (batches ACT table sets):
  ph1 (per chunk): theta = ONE K=8 fp32 matmul (host-packed lhsT/rhs:
         delta-rows give S_ii-S_ij, ones x SflatR gives S_jk,
         (-cT) x cTrep gives -S_ik)  -> PSUM
       denom = 4pi*(d_ij*d_ik+eps)   (GPSIMD tensor_scalar per j)
       recipT = ~1/denom             (DVE reciprocal_approx_fast)
       dlt = |t - round(t)|, t = theta*recipT + 1/4  (custom DVE op) -> fp16
  ph2: c2a = Sin(2pi*dlt) = |cos(theta'/2)|  (trig table, whole molecule)
  ph3 (per chunk): lnw PSUM = bf16 matmuls [ones x lnGflatR + lnG^T x IDrep]
       W = Exp(lnw) = G_ik*G_jk ; l2 = Ln(c2a); p8 = Exp(1.6*l2 + ln2)
       z_j = sum_k p8*W  (tensor_tensor_reduce per j) -> Z[:, j]
  epilogue: out_i = sum_j Z[i,j]*G[i,j]
"""

import numpy as np

# ---- hardcoded problem shape (from spec) ----
B, N = 16, 128
NCORES = 8
MPC = B // NCORES            # molecules per core = 2
EPS = 1e-5
FOURPI = float(4.0 * np.pi)
FOURPI_EPS = float(4.0 * np.pi * EPS)
LN2 = float(np.log(2.0))
MAGIC = 12582912.0           # 1.5*2^23: fp32 round-to-nearest-int magic
TWO_PI_DOWN = float(np.nextafter(np.float32(2.0 * np.pi), np.float32(0.0)))
CHUNK_J = 4                  # j's per PSUM chunk -> mm N = 512
NCHUNK = N // CHUNK_J        # 32
LGCLAMP = -60.0              # clamp for ln(G) (guards dc==0 -> -inf)
KTH = 8                      # K of the fused theta matmul

CFG = {
    "use_custom_round": True,
    "denom_engine": "gpsimd",  # "gpsimd" | "vector"
    "dlt_dtype": "float16",
    "c2a_dtype": "bfloat16",
    "sin_block": 16,           # chunks per Sin instruction
}

_ROUND_OP = None
_GRAPH = None


def _make_round_op():
    """Fused range reduction: out = |t - round(t)|, t = in0*in1 + 0.25.
    Sin(2pi*out) then yields |cos(x)| for x = in0*in1 in turns*2pi."""
    global _ROUND_OP
    if _ROUND_OP is not None:
        return _ROUND_OP
    from concourse import dve_ops
    from concourse.dve_ops import DveOp
    from concourse.dve_spec import C0, C1, Spec, Src0, Src1, Zero, lower, maxx
    from concourse.dve_uop import DveOpSpec

    name = "ANGSYM_RND"
    for op in dve_ops.OPS:
        if op.name == name:
            _ROUND_OP = op
            return op

    tau = Src0 * Src1 + C0
    k = (tau + C1) - C1
    d = tau - k
    body = maxx(d, Zero - d)

    def _ref(in0, in1, s0, s1, imm2):
        f32 = np.float32
        tau = (in0.astype(f32) * in1.astype(f32) + f32(s0)).astype(f32)
        t2 = (tau + f32(s1)).astype(f32)
        kk = (t2 - f32(s1)).astype(f32)
        return np.abs((tau - kk).astype(f32))

    spec = Spec(body=body, reference=_ref)
    opcode = max(dve_ops._SUB_OPCODE_FOR_NAME.values()) + 1
    assert opcode < 0x20
    dve_ops._SUB_OPCODE_FOR_NAME[name] = opcode
    shas = {}
    for ver in ("v3", "v4"):
        try:
            uops = lower(spec, ver=ver)
            shas[ver] = DveOpSpec(
                name=name, opcode=opcode, uops=uops, rd1_en=True
            ).sha(ver)
        except Exception:
            pass
    assert shas, "ANGSYM_RND failed to lower for all DVE versions"
    op = DveOp(name, spec, subdim=False, uops_sha=shas)
    dve_ops.OPS.append(op)
    dve_ops.CUSTOM_DVE_SPECS[name] = spec
    _ROUND_OP = op
    return op


def build_graph(cfg=None):
    """Build the single-core Bass graph (same SPMD graph on all 8 cores)."""
    cfg = dict(CFG, **(cfg or {}))
    from contextlib import ExitStack

    import concourse.bass as bass
    import concourse.tile as tile
    from concourse import bacc, mybir

    f32 = mybir.dt.float32
    bf16 = mybir.dt.bfloat16
    dlt_dt = getattr(mybir.dt, cfg["dlt_dtype"])
    c2a_dt = getattr(mybir.dt, cfg["c2a_dtype"])
    F = mybir.ActivationFunctionType
    ALU = mybir.AluOpType

    from concourse.tile_rust import add_dep_helper

    round_op = _make_round_op() if cfg["use_custom_round"] else None
    assert round_op is not None, "stock round path removed in v2"

    nc = bacc.Bacc()
    # per-molecule host-precomputed feeds
    d_ext = nc.declare_dram_parameter("d", [MPC, N, N], f32, isOutput=False)
    d4pi_ext = nc.declare_dram_parameter("d4pi", [MPC, N, N], f32, isOutput=False)
    g_ext = nc.declare_dram_parameter("G", [MPC, N, N], f32, isOutput=False)
    lhs_ext = nc.declare_dram_parameter(
        "thl", [MPC, 2 * KTH, NCHUNK * N], bf16, isOutput=False
    )
    rhsa_ext = nc.declare_dram_parameter(
        "thra", [MPC, KTH, N * N], bf16, isOutput=False
    )
    rhsb_ext = nc.declare_dram_parameter(
        "thrb", [MPC, 2 * KTH, N * N], bf16, isOutput=False
    )
    lgt_ext = nc.declare_dram_parameter("lgt", [MPC, N, N], bf16, isOutput=False)
    lgf_ext = nc.declare_dram_parameter("lgf", [MPC, 1, N * N], bf16, isOutput=False)
    idrep_ext = nc.declare_dram_parameter(
        "idrep", [N, CHUNK_J * N], bf16, isOutput=False
    )
    out_ext = nc.declare_dram_parameter("out", [MPC, N], f32, isOutput=True)

    SB = CHUNK_J * N  # 512: chunk width

    with ExitStack() as ctx:
        tc = ctx.enter_context(tile.TileContext(nc))
        consts = ctx.enter_context(tc.tile_pool(name="consts", bufs=1))
        molp = ctx.enter_context(tc.tile_pool(name="mol", bufs=1))
        psum_th = ctx.enter_context(
            tc.tile_pool(name="psum_th", bufs=2, space="PSUM")
        )
        psum_lnw = ctx.enter_context(
            tc.tile_pool(name="psum_lnw", bufs=2, space="PSUM")
        )
        work = ctx.enter_context(tc.tile_pool(name="work", bufs=2))
        workbig = ctx.enter_context(tc.tile_pool(name="workbig", bufs=1))
        scrapp = ctx.enter_context(tc.tile_pool(name="scrap", bufs=2))

        # chain every ACT op in program order so the scheduler cannot
        # interleave trig-table and ln/exp-table phases (each switch costs
        # ~1.5us ACT_TABLE_LOAD; unchained we measured 95 loads = 146us)
        _last_act = [None]

        def _chain(ins):
            if _last_act[0] is not None:
                add_dep_helper(
                    ins, _last_act[0], sync=False, reason="act-table-order"
                )
            _last_act[0] = ins

        def act(*a, **kw):
            bi = nc.scalar.activation(*a, **kw)
            _chain(bi.ins)
            return bi

        # the auto table-load pass greedily picks per-function sets
        # (natural_log for Ln, exp_and_others for Exp -> one 1.5us load per
        # transition); pre-load the shared ln+exp set explicitly instead.
        from concourse.hw_specs import get_activation_tables

        _tables = get_activation_tables(nc.m.arch)
        _lnexp_id = next(
            i for i, (nm, fs) in enumerate(_tables.items())
            if F.Ln in fs and F.Exp in fs
        )

        def load_lnexp_table():
            inst = mybir.InstLoadActFuncSet(
                name=nc.get_next_instruction_name(), ins=[], outs=[],
                act_func_set_id=_lnexp_id,
            )
            bi = nc.scalar.add_instruction(inst)
            _chain(bi.ins)

        idrep_sb = consts.tile([N, SB], bf16, tag="idrep")
        nc.sync.dma_start(out=idrep_sb[:], in_=idrep_ext[:])
        ones1b = consts.tile([1, N], bf16, tag="ones1b")
        nc.vector.memset(ones1b[:], 1.0)
        ln2c = consts.tile([N, 1], f32, tag="ln2c")
        nc.vector.memset(ln2c[:], LN2)
        tinyc = consts.tile([N, 1], f32, tag="tinyc")
        nc.vector.memset(tinyc[:], 1e-30)

        for m in range(MPC):
            d_sb = molp.tile([N, N], f32, tag="d_sb")
            nc.sync.dma_start(out=d_sb[:], in_=d_ext[m])
            d4pi = molp.tile([N, N], f32, tag="d4pi")
            nc.sync.dma_start(out=d4pi[:], in_=d4pi_ext[m])
            G = molp.tile([N, N], f32, tag="G")
            nc.sync.dma_start(out=G[:], in_=g_ext[m])
            THL = molp.tile([2 * KTH, NCHUNK * N], bf16, tag="THL")
            nc.sync.dma_start(out=THL[:], in_=lhs_ext[m])
            THRA = molp.tile([KTH, N * N], bf16, tag="THRA")
            nc.sync.dma_start(out=THRA[:], in_=rhsa_ext[m])
            THRB = molp.tile([2 * KTH, N * N], bf16, tag="THRB")
            nc.sync.dma_start(out=THRB[:], in_=rhsb_ext[m])
            LGT = molp.tile([N, N], bf16, tag="LGT")
            nc.sync.dma_start(out=LGT[:], in_=lgt_ext[m])
            LGF = molp.tile([1, N * N], bf16, tag="LGF")
            nc.sync.dma_start(out=LGF[:], in_=lgf_ext[m])

            dlt_all = molp.tile([N, N * N], dlt_dt, tag="dlt_all")
            c2a_all = molp.tile([N, N * N], c2a_dt, tag="c2a_all")
            Z = molp.tile([N, N], f32, tag="Z")

            # ---- phase 1: theta + range-reduce, per chunk ----
            for g in range(NCHUNK):
                js = [g + NCHUNK * q for q in range(CHUNK_J)]
                # theta via split-bf16: hi*hi, then [hi;lo] x [lo;hi]
                TH = psum_th.tile([N, SB], f32, tag="TH")
                nc.tensor.matmul(
                    out=TH[:], lhsT=THL[0:KTH, g * N:(g + 1) * N],
                    rhs=THRA[:, g * SB:(g + 1) * SB], start=True, stop=False,
                )
                nc.tensor.matmul(
                    out=TH[:], lhsT=THL[:, g * N:(g + 1) * N],
                    rhs=THRB[:, g * SB:(g + 1) * SB], start=False, stop=True,
                )
                denom = work.tile([N, SB], f32, tag="denom")
                dng = nc.gpsimd if cfg["denom_engine"] == "gpsimd" else nc.vector
                for q in range(CHUNK_J):
                    dng.tensor_scalar(
                        out=denom[:, q * N:(q + 1) * N], in0=d_sb[:],
                        scalar1=d4pi[:, js[q]:js[q] + 1],
                        scalar2=FOURPI_EPS, op0=ALU.mult, op1=ALU.add,
                    )
                recipT = work.tile([N, SB], f32, tag="recipT")
                nc.vector.reciprocal_approx_fast(out=recipT[:], in_=denom[:])
                nc.vector._custom_dve(
                    round_op, out=dlt_all[:, g * SB:(g + 1) * SB],
                    in0=TH[:], in1=recipT[:], s0=0.25, s1=MAGIC,
                )

            # ---- phase 2: Sin over the whole molecule (trig table) ----
            sb_blk = cfg["sin_block"] * SB
            for o in range(0, N * N, sb_blk):
                act(
                    c2a_all[:, o:o + sb_blk], dlt_all[:, o:o + sb_blk],
                    F.Sin, bias=0.0, scale=TWO_PI_DOWN,
                )

            # ---- phase 3: weights + pow + reduce (ln/exp table) ----
            # Ln/Exp run on 2-chunk blocks; lnw matmuls + ExpW per chunk.
            load_lnexp_table()
            for gb in range(0, NCHUNK, 4):
                l2 = workbig.tile([N, 4 * SB], bf16, tag="l2")
                act(
                    l2[:], c2a_all[:, gb * SB:(gb + 4) * SB], F.Ln,
                    bias=tinyc[:],
                )
                p8 = workbig.tile([N, 4 * SB], f32, tag="p8")
                act(p8[:], l2[:], F.Exp, bias=ln2c[:], scale=1.6)
                for g in range(gb, gb + 4):
                    js = [g + NCHUNK * q for q in range(CHUNK_J)]
                    LNW = psum_lnw.tile([N, SB], f32, tag="LNW")
                    nc.tensor.matmul(
                        out=LNW[:], lhsT=ones1b[:],
                        rhs=LGF[:, g * SB:(g + 1) * SB], start=True, stop=False,
                    )
                    nc.tensor.matmul(
                        out=LNW[:], lhsT=LGT[:], rhs=idrep_sb[:],
                        start=False, stop=True,
                    )
                    W = work.tile([N, SB], f32, tag="W")
                    act(W[:], LNW[:], F.Exp)
                    po = (g - gb) * SB
                    for q in range(CHUNK_J):
                        scrap = scrapp.tile([N, N], f32, tag="scrap")
                        nc.vector.affine_mul_reduce(
                            out=scrap[:], accum_out=Z[:, js[q]:js[q] + 1],
                            in0=p8[:, po + q * N:po + (q + 1) * N],
                            in1=W[:, q * N:(q + 1) * N],
                            scale=1.0, bias=0.0,
                        )

            # ---- epilogue: out_i = sum_j Z[i,j]*G[i,j] ----
            outc = molp.tile([N, 1], f32, tag="outc")
            escrap = scrapp.tile([N, N], f32, tag="escrap")
            nc.vector.affine_mul_reduce(
                out=escrap[:], accum_out=outc[:], in0=Z[:], in1=G[:],
                scale=1.0, bias=0.0,
            )
            nc.sync.dma_start(out=out_ext[m], in_=outc[:])

    return nc


def _get_graph():
    global _GRAPH
    if _GRAPH is None:
        _GRAPH = build_graph()
        _GRAPH.finalize()
    return _GRAPH


def _host_precompute(d, dc, coords):
    """Per-molecule numpy precompute of the packed device feeds.
    d, dc: [N, N] f32;  coords: [N, 3] f32.  All O(N^2)."""
    import ml_dtypes

    f32 = np.float32
    C = coords.astype(np.float64)
    S = (C @ C.T).astype(f32)                      # Gram
    diag = np.diag(S).copy()
    lnG = np.maximum(
        np.log(dc.astype(np.float64) + 1e-30) - d.astype(np.float64) ** 2,
        LGCLAMP,
    ).astype(f32)
    G = np.exp(lnG).astype(f32)
    cT = coords.T.astype(f32)                      # [3, N]

    # theta matmul masters; chunk g covers js = g + 32*q
    # lhsT slice at col g*128; rhs slice at col g*512
    THL = np.zeros((KTH, NCHUNK * N), f32)
    THR = np.zeros((KTH, N * N), f32)
    for g in range(NCHUNK):
        li = slice(g * N, (g + 1) * N)
        ri = slice(g * CHUNK_J * N, (g + 1) * CHUNK_J * N)
        lhs = np.zeros((KTH, N), f32)
        rhs = np.zeros((KTH, CHUNK_J * N), f32)
        for q in range(CHUNK_J):
            j = g + NCHUNK * q
            lhs[q, :] = diag - S[:, j]             # S_ii - S_ij  (per i)
            rhs[q, q * N:(q + 1) * N] = 1.0        # delta row
            rhs[4, q * N:(q + 1) * N] = S[j, :]    # S_jk
            rhs[5:8, q * N:(q + 1) * N] = cT       # cTrep
        lhs[4, :] = 1.0                            # ones row (pairs S_jk)
        lhs[5:8, :] = -cT                          # -S_ik
        THL[:, li] = lhs
        THR[:, ri] = rhs

    # split-bf16: a = hi + lo; theta = hi.hi + (hi.lo + lo.hi); the lo.lo
    # term (~2^-16 relative) is dropped.
    def split_bf16(a):
        hi = a.astype(ml_dtypes.bfloat16)
        lo = (a - hi.astype(f32)).astype(ml_dtypes.bfloat16)
        return hi, lo

    THL_hi, THL_lo = split_bf16(THL)
    THR_hi, THR_lo = split_bf16(THR)
    thl = np.concatenate([THL_hi, THL_lo], axis=0)     # [16, 4096] (hi; lo)
    thra = THR_hi                                      # [8, 16384]
    thrb = np.concatenate([THR_lo, THR_hi], axis=0)    # [16, 16384] (lo; hi)

    lgt = np.ascontiguousarray(lnG.T).astype(ml_dtypes.bfloat16)
    # lgf[0, g*512 + q*128 + k] = lnG[g+32q, k]
    lgf = np.zeros((1, N * N), np.float32)
    for q in range(CHUNK_J):
        rows = lnG[NCHUNK * q:NCHUNK * (q + 1), :]          # [32, 128]
        lgf[0].reshape(NCHUNK, CHUNK_J, N)[:, q, :] = rows
    lgf = lgf.astype(ml_dtypes.bfloat16)
    return {
        "d": d.astype(f32),
        "d4pi": (FOURPI * d).astype(f32),
        "G": G,
        "thl": thl,
        "thra": thra,
        "thrb": thrb,
        "lgt": lgt,
        "lgf": lgf,
    }


def make_in_maps(d_cutoff, d, atom_coordinates):
    import ml_dtypes

    idrep = np.ascontiguousarray(
        np.tile(np.eye(N, dtype=np.float32), (1, CHUNK_J))
    ).astype(ml_dtypes.bfloat16)
    in_maps = []
    for c in range(NCORES):
        per_mol = [
            _host_precompute(
                np.asarray(d[c * MPC + m], dtype=np.float32),
                np.asarray(d_cutoff[c * MPC + m], dtype=np.float32),
                np.asarray(atom_coordinates[c * MPC + m], dtype=np.float32),
            )
            for m in range(MPC)
        ]
        im = {
            k: np.ascontiguousarray(np.stack([pm[k] for pm in per_mol]))
            for k in per_mol[0]
        }
        im["idrep"] = idrep
        in_maps.append(im)
    return in_maps


def kernel(d_cutoff, d, atom_coordinates):
    from concourse.bass_utils import run_bass_kernel_spmd

    nc = _get_graph()
    in_maps = make_in_maps(d_cutoff, d, atom_coordinates)
    res = run_bass_kernel_spmd(nc, in_maps, list(range(NCORES)))
    out = np.concatenate(
        [res.results[i]["out"] for i in range(NCORES)], axis=0
    ).astype(np.float32)
    return out
